# revision 1
# baseline (speedup 1.0000x reference)
"""GCN encoder (2-layer, mu/logstd heads) on 8 Trainium2 NeuronCores.

Strategy v2 (aggregate-then-project, 1D dst-partitioned graph):
  - Host: add self-loops, fold the FULL symmetric normalization into per-edge
    weights (ew = deg^-1/2[s] * w * deg^-1/2[d], f64), build a load-balancing
    node permutation (deal by in-degree into 392 blocks of 128 lanes over 8
    cores), and ONE shared edge layout used by both passes: every edge slotted
    into (core, block, lo/hi, tile, partition); wrapped-16 int16 SWDGE gather
    indices against PERMUTED node tables; per-slot dst-lane + edge-weight.
    Stage xperm = x rows permuted to table order (bf16).
  - Device (single SPMD program, TileContext):
      Pass 1: per window: dma_gather RAW x rows (bf16) -> edge-major tiles;
              omega[e,n] = (iota==dst_lane)*ew via one dual-op tensor_scalar
              per tile; PE matmuls accumulate agg[n,:] = sum_e ew*x[s] per
              128-node block in PSUM; then per block PROJECT ONCE:
              transpose(agg) -> matmul W1 -> +b1 -> ReLU -> h1 slab (SBUF,
              node-major); DMA h1 block to ag_in as soon as it's ready.
      AllGather the 8 h1 shards -> table2 (bf16, node-major, permuted order).
      Pass 2: identical windows/slots/omega against table2; per block:
              project agg2 by [Wmu||Wls] + bias, f32 out, split mu/ls DMA.
  - Host: inverse-permute rows, return (mu, logstd).

All normalization lives in ew; no per-node scaling on device. Both passes
share one index/dst/ew slab set (identical gather geometry, IN_CH==HID).
"""

import os
import sys

sys.path.insert(0, "/opt/trn_rl_repo")

import numpy as np
import ml_dtypes
from contextlib import ExitStack

import concourse.bass as bass
import concourse.bacc as bacc
import concourse.mybir as mybir
import concourse.tile as tile
from concourse.bass_utils import run_bass_kernel_spmd

P = 128
NCORES = 8
VLO = 32768          # int16 index range per gather table view
WINDOW_BLOCKS = int(os.environ.get("KERNEL_WB", "4"))

BF16 = mybir.dt.bfloat16
F32 = mybir.dt.float32
I16 = mybir.dt.int16
NPBF16 = ml_dtypes.bfloat16


def _ceil_div(a, b):
    return -(-a // b)


# ----------------------------------------------------------------------------
# Host preprocessing
# ----------------------------------------------------------------------------

def _build_pass_layout(src_rows, e_core, e_brow, e_lane, e_ew, nblk, n_table_rows):
    """Slot every edge into (core, block, class, tile, partition); produce
    wrapped-16 int16 index slabs and per-slot dst-lane / edge-weight."""
    n_edges = len(src_rows)
    is_lo = src_rows < VLO
    gid = (e_core * nblk + e_brow) * 2 + (~is_lo).astype(np.int64)
    # secondary sort by src row: consecutive gather descriptors hit ascending
    # table addresses (DRAM row-buffer locality)
    order = np.argsort(gid * (1 << 17) + src_rows, kind="stable")
    gid_s = gid[order]
    counts = np.bincount(gid_s, minlength=NCORES * nblk * 2)
    starts = np.concatenate([[0], np.cumsum(counts)[:-1]])
    rank = np.arange(n_edges) - starts[gid_s]

    cnt_lo = counts[0::2].reshape(NCORES, nblk)
    cnt_hi = counts[1::2].reshape(NCORES, nblk)
    K_LO = max(1, int(_ceil_div(cnt_lo.max(), P)))
    K_HI = int(_ceil_div(cnt_hi.max(), P)) if cnt_hi.max() > 0 else 0
    K = K_LO + K_HI

    windows = []
    b = 0
    while b < nblk:
        wb = min(WINDOW_BLOCKS, nblk - b)
        windows.append((b, wb))
        b += wb

    # global tile index: window w holds [lo tiles of its wb blocks][hi tiles]
    tile_base = np.zeros(nblk, np.int64)
    win_of_brow = np.zeros(nblk, np.int64)
    j_of_brow = np.zeros(nblk, np.int64)
    wb_of_brow = np.zeros(nblk, np.int64)
    base = 0
    for w, (b0, wb) in enumerate(windows):
        for j in range(wb):
            tile_base[b0 + j] = base
            win_of_brow[b0 + j] = w
            j_of_brow[b0 + j] = j
            wb_of_brow[b0 + j] = wb
        base += wb * K
    TOT_TILES = base

    e_core_s = e_core[order]
    e_brow_s = e_brow[order]
    e_lane_s = e_lane[order]
    e_ew_s = e_ew[order]
    src_s = src_rows[order]
    is_lo_s = is_lo[order]

    k_local = rank // P
    p_slot = rank % P
    wb_s = wb_of_brow[e_brow_s]
    j_s = j_of_brow[e_brow_s]
    t_in_w = np.where(is_lo_s, j_s * K_LO + k_local,
                      wb_s * K_LO + j_s * K_HI + k_local)
    gt = tile_base[e_brow_s] + t_in_w

    dst_slab = np.full((NCORES, P, TOT_TILES), -1.0, np.float32)
    ew_slab = np.zeros((NCORES, P, TOT_TILES), np.float32)
    idx32_slab = np.zeros((NCORES, P, TOT_TILES), np.int32)
    dst_slab[e_core_s, p_slot, gt] = e_lane_s.astype(np.float32)
    ew_slab[e_core_s, p_slot, gt] = e_ew_s.astype(np.float32)
    idx32_slab[e_core_s, p_slot, gt] = src_s.astype(np.int32)

    lo_cols_per_win = [wb * K_LO * P // 16 for (_, wb) in windows]
    hi_cols_per_win = [wb * K_HI * P // 16 for (_, wb) in windows]
    lo_col_base = np.concatenate([[0], np.cumsum(lo_cols_per_win)[:-1]]).astype(np.int64)
    hi_col_base = np.concatenate([[0], np.cumsum(hi_cols_per_win)[:-1]]).astype(np.int64)
    lo_idx = np.zeros((NCORES, 16, int(sum(lo_cols_per_win))), np.int16)
    hi_idx = np.zeros((NCORES, 16, max(1, int(sum(hi_cols_per_win)))), np.int16)

    flat_in_region = np.where(
        is_lo_s,
        (j_s * K_LO + k_local) * P + p_slot,
        (j_s * K_HI + k_local) * P + p_slot,
    )
    w_s = win_of_brow[e_brow_s]
    col = np.where(is_lo_s, lo_col_base[w_s], hi_col_base[w_s]) + flat_in_region // 16
    row = flat_in_region % 16
    lo_mask = is_lo_s
    lo_idx[e_core_s[lo_mask], row[lo_mask], col[lo_mask]] = src_s[lo_mask].astype(np.int16)
    if K_HI > 0:
        hi_mask = ~is_lo_s
        hi_idx[e_core_s[hi_mask], row[hi_mask], col[hi_mask]] = (
            (src_s[hi_mask] - VLO).astype(np.int16))

    return dict(
        K_LO=K_LO, K_HI=K_HI, K=K, TOT_TILES=TOT_TILES, windows=windows,
        dst_slab=dst_slab, ew_slab=ew_slab, idx32_slab=idx32_slab,
        lo_idx=np.tile(lo_idx, (1, 8, 1)), hi_idx=np.tile(hi_idx, (1, 8, 1)),
        lo_col_base=lo_col_base, hi_col_base=hi_col_base,
        n_table_rows=n_table_rows,
    )


def _preprocess(x, edge_index, weight):
    N = x.shape[0]
    s = edge_index[0].astype(np.int64)
    d = edge_index[1].astype(np.int64)
    w = weight.astype(np.float64)
    s = np.concatenate([s, np.arange(N)])
    d = np.concatenate([d, np.arange(N)])
    w = np.concatenate([w, np.ones(N)])

    deg = np.bincount(d, weights=w, minlength=N)
    dis = np.where(deg > 0, deg ** -0.5, 0.0)
    ew = dis[s] * w * dis[d]          # full symmetric norm folded into ew

    NB = NCORES * _ceil_div(_ceil_div(N, NCORES), P)
    nblk = NB // NCORES
    PAD_CORE = nblk * P
    PAD_N = NB * P

    # balance: round-robin deal nodes (sorted by in-degree desc) into NB blocks
    tot = np.bincount(d, minlength=N)
    order = np.argsort(-tot, kind="stable")
    blk = np.empty(N, np.int64)
    lane = np.empty(N, np.int64)
    blk[order] = np.arange(N) % NB
    lane[order] = np.arange(N) // NB
    assert lane.max() < P
    core_of = blk // nblk
    brow_of = blk % nblk
    permpos = core_of * PAD_CORE + brow_of * P + lane

    # chunk-major table layout so each AllGather chunk lands contiguously:
    # row(core, brow, lane) = chunk*8*CB*P + core*CB*P + (brow%CB)*P + lane
    CB = int(os.environ.get("KERNEL_CB", "7"))
    nchunk = _ceil_div(nblk, CB)
    RPC = NCORES * CB * P           # table rows per chunk
    chunk_of = brow_of // CB
    permtab = (chunk_of * RPC + core_of * CB * P
               + (brow_of % CB) * P + lane)
    PAD_T = nchunk * RPC            # padded table rows (>= PAD_N)

    e_core = core_of[d]
    e_brow = brow_of[d]
    e_lane = lane[d]

    pl = _build_pass_layout(permtab[s], e_core, e_brow, e_lane, ew, nblk, PAD_T)

    return dict(
        N=N, NB=NB, nblk=nblk, PAD_CORE=PAD_CORE, PAD_N=PAD_N,
        permpos=permpos, permtab=permtab, CB=CB, nchunk=nchunk, RPC=RPC,
        PAD_T=PAD_T, pl=pl,
    )


# ----------------------------------------------------------------------------
# Device program
# ----------------------------------------------------------------------------

def _emit_pass(nc, pools, pl, table_dram, lo_s, hi_s,
               dst_s, ew_s, iota_s, flush_fn, ix32_s=None):
    abl = os.environ.get("KERNEL_ABL", "")
    gmode = os.environ.get("KERNEL_GMODE", "swdge")
    K_LO, K_HI, K = pl["K_LO"], pl["K_HI"], pl["K"]
    windows = pl["windows"]
    lo_col_base, hi_col_base = pl["lo_col_base"], pl["hi_col_base"]
    rows = pl["n_table_rows"]
    msg_pool, omega_pool, psum_pool = pools["msg"], pools["omega"], pools["psum"]
    nq = int(os.environ.get("KERNEL_NSWQ", "2"))

    max_wb = max(wb for _, wb in windows)
    tbl_lo = table_dram[0:VLO, :]
    tbl_hi = table_dram[VLO:rows, :] if rows > VLO else None

    for w, (b0, wb) in enumerate(windows):
        wtiles = wb * K
        nlo_tiles = wb * K_LO
        msg = msg_pool.tile([P, max_wb * K, P], BF16, tag="msg")
        omega = omega_pool.tile([P, max_wb * K * P], BF16, tag="omega")
        n_lo = wb * K_LO * P
        if "nogather" in abl:
            pass
        elif gmode == "ind":
            for t in range(wtiles):
                gt = b0 * K + t
                nc.gpsimd.indirect_dma_start(
                    out=msg[:, t, :], out_offset=None,
                    in_=table_dram[:],
                    in_offset=bass.IndirectOffsetOnAxis(
                        ap=ix32_s[:, gt:gt + 1], axis=0))
        else:
            nc.gpsimd.dma_gather(
                out_ap=msg[:, 0:nlo_tiles, :],
                in_ap=tbl_lo,
                idxs_ap=lo_s[:, int(lo_col_base[w]):int(lo_col_base[w]) + n_lo // 16],
                num_idxs=n_lo,
                num_idxs_reg=n_lo,
                elem_size=P,
                queue_num=(2 * w) % nq,
                single_packet=(n_lo <= 1024),
            )
            if K_HI > 0:
                n_hi = wb * K_HI * P
                nc.gpsimd.dma_gather(
                    out_ap=msg[:, nlo_tiles:nlo_tiles + wb * K_HI, :],
                    in_ap=tbl_hi,
                    idxs_ap=hi_s[:, int(hi_col_base[w]):int(hi_col_base[w]) + n_hi // 16],
                    num_idxs=n_hi,
                    num_idxs_reg=n_hi,
                    elem_size=P,
                    queue_num=(2 * w + 1) % nq,
                    single_packet=(n_hi <= 1024),
                )
        gt0 = b0 * K
        if "noomega" not in abl:
            for t in range(wtiles):
                nc.vector.tensor_scalar(
                    out=omega[:, t * P:(t + 1) * P],
                    in0=iota_s,
                    scalar1=dst_s[:, gt0 + t:gt0 + t + 1],
                    scalar2=ew_s[:, gt0 + t:gt0 + t + 1],
                    op0=mybir.AluOpType.is_equal,
                    op1=mybir.AluOpType.mult,
                )
        if "noflush" in abl and "nomm" in abl:
            continue
        for j in range(wb):
            brow = b0 + j
            acc = psum_pool.tile([P, P], F32, tag="acc", space="PSUM")
            if "nomm" not in abl:
                for k in range(K_LO):
                    t = j * K_LO + k
                    nc.tensor.matmul(
                        out=acc[:], lhsT=omega[:, t * P:(t + 1) * P],
                        rhs=msg[:, t, :], start=(k == 0), stop=(k == K - 1 and K_HI == 0))
                for k in range(K_HI):
                    t = wb * K_LO + j * K_HI + k
                    nc.tensor.matmul(
                        out=acc[:], lhsT=omega[:, t * P:(t + 1) * P],
                        rhs=msg[:, nlo_tiles + j * K_HI + k, :],
                        start=False, stop=(k == K_HI - 1))
            else:
                nc.tensor.matmul(out=acc[:], lhsT=iota_s, rhs=iota_s,
                                 start=True, stop=True)
            if "noflush" not in abl:
                flush_fn(brow, acc)


def _build_program(meta, IN_CH, HID, OUT):
    pl = meta["pl"]
    nblk = meta["nblk"]
    PAD_CORE, PAD_N = meta["PAD_CORE"], meta["PAD_N"]
    HOUT = 2 * OUT
    abl = os.environ.get("KERNEL_ABL", "")

    nq = int(os.environ.get("KERNEL_NSWQ", "2"))
    scratch = int(os.environ.get("KERNEL_SCRATCH", "16384"))
    nc = bacc.Bacc(num_swdge_queues=nq, dynamic_dma_scratch_size=scratch)
    xp_t = nc.declare_dram_parameter("xperm", [meta["PAD_T"], IN_CH], BF16, isOutput=False)
    W1_t = nc.declare_dram_parameter("W1", [P, HID], BF16, isOutput=False)
    Wcat_t = nc.declare_dram_parameter("Wcat", [HID, HOUT], BF16, isOutput=False)
    b1_t = nc.declare_dram_parameter("b1", [1, HID], BF16, isOutput=False)
    bcat_t = nc.declare_dram_parameter("bcat", [1, HOUT], BF16, isOutput=False)
    iota_t = nc.declare_dram_parameter("iota", [P, P], BF16, isOutput=False)

    lo_t = nc.declare_dram_parameter("lo", [P, pl["lo_idx"].shape[2]], I16, isOutput=False)
    hi_t = nc.declare_dram_parameter("hi", [P, pl["hi_idx"].shape[2]], I16, isOutput=False)
    dst_t = nc.declare_dram_parameter("dst", [P, pl["TOT_TILES"]], F32, isOutput=False)
    ew_t = nc.declare_dram_parameter("ew", [P, pl["TOT_TILES"]], F32, isOutput=False)
    gmode = os.environ.get("KERNEL_GMODE", "swdge")
    ix32_t = (nc.declare_dram_parameter("ix32", [P, pl["TOT_TILES"]],
                                        mybir.dt.int32, isOutput=False)
              if gmode == "ind" else None)

    mu_t = nc.declare_dram_parameter("mu", [PAD_CORE, OUT], F32, isOutput=True)
    ls_t = nc.declare_dram_parameter("ls", [PAD_CORE, OUT], F32, isOutput=True)

    CB, nchunk, RPC = meta["CB"], meta["nchunk"], meta["RPC"]
    PAD_T = meta["PAD_T"]
    ag_ins = [
        nc.dram_tensor(f"ag_in{k}", [min(CB, nblk - k * CB) * P, HID], BF16)
        for k in range(nchunk)
    ]
    table2 = nc.dram_tensor("table2", [PAD_T, HID], BF16, addr_space="Shared")

    with tile.TileContext(nc) as tc, ExitStack() as ctx:
        const = ctx.enter_context(tc.tile_pool(name="const", bufs=1))
        stage_pool = ctx.enter_context(tc.tile_pool(name="stage", bufs=3))
        msg_pool = ctx.enter_context(tc.tile_pool(name="msg", bufs=2))
        omega_pool = ctx.enter_context(tc.tile_pool(name="omega", bufs=2))
        psum_pool = ctx.enter_context(tc.tile_pool(name="psum", bufs=3, space="PSUM"))
        pr_pool = ctx.enter_context(tc.tile_pool(name="prpsum", bufs=2, space="PSUM"))
        tp_pool = ctx.enter_context(tc.tile_pool(name="tpsum", bufs=2, space="PSUM"))

        def load_const(param, shape, dtype):
            s = const.tile(shape, dtype, tag=param.name)
            nc.sync.dma_start(out=s[:], in_=param[:])
            return s[:]

        W1_s = load_const(W1_t, [P, HID], BF16)
        Wcat_s = load_const(Wcat_t, [HID, HOUT], BF16)
        b1_s = load_const(b1_t, [1, HID], BF16)
        bcat_s = load_const(bcat_t, [1, HOUT], BF16)
        iota_s = load_const(iota_t, [P, P], BF16)
        lo_s = load_const(lo_t, [P, pl["lo_idx"].shape[2]], I16)
        hi_s = load_const(hi_t, [P, pl["hi_idx"].shape[2]], I16)
        dst_s = load_const(dst_t, [P, pl["TOT_TILES"]], F32)
        ew_s = load_const(ew_t, [P, pl["TOT_TILES"]], F32)
        ix32_s = (load_const(ix32_t, [P, pl["TOT_TILES"]], mybir.dt.int32)
                  if ix32_t is not None else None)

        ones_s = const.tile([1, P], BF16, tag="ones")
        nc.vector.memset(ones_s[:], 1.0)
        identity_s = const.tile([P, P], BF16, tag="identity")
        nc.vector.memset(identity_s[:], 0.0)
        nc.gpsimd.affine_select(
            out=identity_s[:], in_=identity_s[:],
            compare_op=mybir.AluOpType.not_equal, fill=1.0,
            base=0, pattern=[[-1, P]], channel_multiplier=1)

        h1 = const.tile([P, nblk * HID], BF16, tag="h1")

        pools = dict(msg=msg_pool, omega=omega_pool, psum=psum_pool)

        def project_block(acc, Ws, bias_s, width):
            """PSUM agg [P,P] -> transpose -> @Ws + bias -> PSUM [P,width]."""
            c = stage_pool.tile([P, P], BF16, tag="pb_c")
            nc.scalar.copy(out=c[:], in_=acc[:])
            tp = tp_pool.tile([P, P], BF16, tag="pb_tp", space="PSUM")
            nc.tensor.transpose(out=tp[:], in_=c[:], identity=identity_s)
            cT = stage_pool.tile([P, P], BF16, tag="pb_cT")
            nc.scalar.copy(out=cT[:], in_=tp[:])
            pr = pr_pool.tile([P, width], F32, tag="pb_pr", space="PSUM")
            nc.tensor.matmul(out=pr[:], lhsT=cT[:], rhs=Ws, start=True, stop=False)
            nc.tensor.matmul(out=pr[:], lhsT=ones_s[:], rhs=bias_s,
                             start=False, stop=True)
            return pr

        def emit_chunk_ag(k):
            if "noAG" in abl:
                return
            rows_k = min(CB, nblk - k * CB) * P
            nc.gpsimd.collective_compute(
                "AllGather", mybir.AluOpType.bypass,
                replica_groups=[list(range(NCORES))],
                ins=[ag_ins[k][:]],
                outs=[table2[k * RPC:k * RPC + rows_k * NCORES, :]])

        def flush1(brow, acc):
            pr = project_block(acc, W1_s, b1_s, HID)
            nc.scalar.activation(out=h1[:, brow * HID:(brow + 1) * HID], in_=pr[:],
                                 func=mybir.ActivationFunctionType.Relu)
            k, r = brow // CB, brow % CB
            nc.sync.dma_start(out=ag_ins[k][r * P:(r + 1) * P, :],
                              in_=h1[:, brow * HID:(brow + 1) * HID])
            if brow == nblk - 1 or r == CB - 1:
                emit_chunk_ag(k)

        if "noB" not in abl:
            _emit_pass(nc, pools, pl, xp_t, lo_s, hi_s, dst_s, ew_s, iota_s, flush1, ix32_s=ix32_s)
        else:
            nc.vector.memset(h1[:], 0.1)
            for brow in range(nblk):
                k, r = brow // CB, brow % CB
                nc.sync.dma_start(out=ag_ins[k][r * P:(r + 1) * P, :],
                                  in_=h1[:, brow * HID:(brow + 1) * HID])
                if brow == nblk - 1 or r == CB - 1:
                    emit_chunk_ag(k)

        tc.strict_bb_all_engine_barrier()

        def flush2(brow, acc):
            pr = project_block(acc, Wcat_s, bcat_s, HOUT)
            o = stage_pool.tile([P, HOUT], F32, tag="otile")
            nc.scalar.copy(out=o[:], in_=pr[:])
            nc.sync.dma_start(out=mu_t[brow * P:(brow + 1) * P, :], in_=o[:, 0:OUT])
            nc.sync.dma_start(out=ls_t[brow * P:(brow + 1) * P, :], in_=o[:, OUT:HOUT])

        if "noD" not in abl:
            _emit_pass(nc, pools, pl, table2, lo_s, hi_s, dst_s, ew_s, iota_s, flush2, ix32_s=ix32_s)

    nc.finalize()
    return nc


# ----------------------------------------------------------------------------
# Public entry
# ----------------------------------------------------------------------------

def _prepare(x, edge_index, weight, W1, b1, Wmu, bmu, Wls, bls):
    x = np.asarray(x)
    N, IN_CH = x.shape
    HID = np.asarray(W1).shape[1]
    OUT = np.asarray(Wmu).shape[1]
    meta = _preprocess(x, np.asarray(edge_index), np.asarray(weight))
    pl = meta["pl"]

    nc = _build_program(meta, IN_CH, HID, OUT)

    xperm = np.zeros((meta["PAD_T"], IN_CH), np.float32)
    xperm[meta["permtab"]] = np.asarray(x, np.float32)
    Wcat = np.concatenate([np.asarray(Wmu), np.asarray(Wls)], axis=1)
    bcat = np.concatenate([np.asarray(bmu), np.asarray(bls)])
    iota = np.tile(np.arange(P, dtype=np.float32)[None, :], (P, 1))

    common = {
        "xperm": xperm.astype(NPBF16),
        "W1": np.asarray(W1, np.float32).astype(NPBF16),
        "Wcat": Wcat.astype(np.float32).astype(NPBF16),
        "b1": np.asarray(b1, np.float32).astype(NPBF16)[None, :],
        "bcat": bcat.astype(np.float32).astype(NPBF16)[None, :],
        "iota": iota.astype(NPBF16),
    }
    in_maps = []
    for c in range(NCORES):
        m = dict(common)
        m["lo"] = pl["lo_idx"][c]
        m["hi"] = pl["hi_idx"][c]
        if os.environ.get("KERNEL_GMODE", "swdge") == "ind":
            m["ix32"] = pl["idx32_slab"][c]
        m["dst"] = pl["dst_slab"][c]
        m["ew"] = pl["ew_slab"][c]
        in_maps.append(m)
    return nc, in_maps, meta


def _postprocess(results, meta):
    mu_cat = np.concatenate([results[c]["mu"] for c in range(NCORES)])
    ls_cat = np.concatenate([results[c]["ls"] for c in range(NCORES)])
    mu = mu_cat[meta["permpos"]].astype(np.float32)
    ls = ls_cat[meta["permpos"]].astype(np.float32)
    return mu, ls


def _run(x, edge_index, weight, W1, b1, Wmu, bmu, Wls, bls, trace=False):
    nc, in_maps, meta = _prepare(x, edge_index, weight, W1, b1, Wmu, bmu, Wls, bls)
    res = run_bass_kernel_spmd(nc, in_maps, list(range(NCORES)), trace=trace)
    return _postprocess(res.results, meta), res


def kernel(x, edge_index, weight, W1, b1, Wmu, bmu, Wls, bls):
    (mu, ls), _ = _run(x, edge_index, weight, W1, b1, Wmu, bmu, Wls, bls)
    return mu, ls

